# revision 4
# baseline (speedup 1.0000x reference)
"""AutoCorrelation Trainium2 kernel (Bass/Tile, 8 NeuronCores) — v3.

Math (per row r of [B*L, 512] with D=512):
  corr_r = irfft(rfft(q_r) * conj(rfft(k_r)))            (circular cross-correlation)
  mean_r = mean(top7(corr_r))
  w0 = sigmoid(corr - mean); out = v*w0 + roll(v,-1,L)*(1-w0)
     = v + sigmoid(mean - corr) * (roll(v) - v)

v3: re-blocked for measured per-instruction overheads:
  - PE matmul ~= N*0.417ns + 15..50ns with enough PSUM banks; PSUM-slot reuse
    stalls (~120ns/mm) dominate at small N -> use N=512 4-accum chains and
    bank-alternating rings.
  - ACT op ~= 460ns fixed + 0.83ns/elem -> merge copies into wide [128,1024+]
    ops, split across ACT/DVE.
  - Pool ALU 1.73ns/elem -> only the final combine add/mul live there.
  - Transposes on PE (f16 is_transpose chunks into PSUM), NOT the DMA xbar:
    fabric cap ~320-340 GB/s/core is fully consumed by the mandatory HBM I/O.
  - Processing unit = half-superblock (4 subblocks = 512 rows).
"""
import numpy as np

B, L, D = 32, 2048, 512
N_CORES = 8
BPC = B // N_CORES            # batches per core
ROWS = BPC * L                # 8192 rows per core
NSUB = 64                     # subblocks (s = row % 64)
P = 128                       # partitions (p = row // 64)
SB_GROUP = 8                  # subblocks per DMA superblock
NSUPER = NSUB // SB_GROUP     # 8 superblocks
TOPK = 7

_CACHE = {}


def _dft_consts():
    """Packed-real DFT matrices W [512 feat, 512 packed] and C [512 packed, 512 t]."""
    j = np.arange(D)[:, None].astype(np.float64)
    f = np.arange(256)[None, :].astype(np.float64)
    Wc = np.cos(-2 * np.pi * j * f / D)
    Ws = np.sin(-2 * np.pi * j * f / D)
    WB = Ws.copy()
    WB[:, 0] = np.cos(np.pi * j[:, 0])          # B0 row: Re256
    W = np.concatenate([Wc, WB], axis=1)        # [512, 512]
    t = np.arange(D)[None, :].astype(np.float64)
    fc = np.arange(256)[:, None].astype(np.float64)
    Ca = np.cos(2 * np.pi * fc * t / D) * 2 / D
    Ca[0] = 1.0 / D
    Cb = -np.sin(2 * np.pi * fc * t / D) * 2 / D
    Cb[0] = np.cos(np.pi * t[0]) / D
    C = np.concatenate([Ca, Cb], axis=0)        # [512, 512]
    return W.astype(np.float32), C.astype(np.float32)


def _build_nc(n_iter=1, internal_io=False):
    import concourse.bacc as bacc
    import concourse.mybir as mybir
    from concourse.tile import TileContext

    f16 = mybir.dt.float16
    f32 = mybir.dt.float32

    W, C = _dft_consts()
    # W16[p, jj, fp]  = W[jj*128+p, fp]   (lhsT blocks for GEMM-1)
    W16 = W.reshape(4, P, D).transpose(1, 0, 2).astype(np.float16).copy()
    # C16[p, ff, t]   = C[ff*128+p, t]    (rhs blocks for GEMM-2)
    C16 = C.reshape(4, P, D).transpose(1, 0, 2).astype(np.float16).copy()
    ID16 = np.eye(P, dtype=np.float16)

    nc = bacc.Bacc()
    tick_d = tock_d = None
    if internal_io:
        tick_d = nc.dram_tensor("tick", [1, 64], f32, kind="ExternalInput")
        tock_d = nc.dram_tensor("tock", [1, 64], f32, kind="ExternalOutput")
        q_d = nc.dram_tensor("query", [ROWS, D], f32, kind="Internal")
        k_d = nc.dram_tensor("key", [ROWS, D], f32, kind="Internal")
        v_d = nc.dram_tensor("value", [ROWS, D], f32, kind="Internal")
        o_d = nc.dram_tensor("out", [ROWS, D], f16, kind="Internal")
    else:
        q_d = nc.dram_tensor("query", [ROWS, D], f32, kind="ExternalInput")
        k_d = nc.dram_tensor("key", [ROWS, D], f32, kind="ExternalInput")
        v_d = nc.dram_tensor("value", [ROWS, D], f32, kind="ExternalInput")
        o_d = nc.dram_tensor("out", [ROWS, D], f16, kind="ExternalOutput")
    w_t = nc.inline_tensor(W16, name="Wdft")
    c_t = nc.inline_tensor(C16, name="Cdft")
    i_t = nc.inline_tensor(ID16, name="Ident")

    qv = q_d.rearrange("(p s) c -> p s c", s=NSUB)
    kv = k_d.rearrange("(p s) c -> p s c", s=NSUB)
    vv = v_d.rearrange("(p s) c -> p s c", s=NSUB)
    ov = o_d.rearrange("(p s) c -> p s c", s=NSUB)

    with TileContext(nc) as tc:
        with (
            tc.tile_pool(name="consts", bufs=1) as consts,
            tc.tile_pool(name="io", bufs=2) as io,
            tc.tile_pool(name="work", bufs=2) as work,
            tc.tile_pool(name="small", bufs=8) as small,
            tc.tile_pool(name="psx", bufs=3, space="PSUM") as psxp,   # 3x1 bank
            tc.tile_pool(name="psg1", bufs=3, space="PSUM") as psg1,  # 3x1 bank
            tc.tile_pool(name="psg2", bufs=2, space="PSUM") as psg2,  # 2x1 bank
        ):
            if internal_io:
                tkt = consts.tile([1, 64], f32)
                nc.sync.dma_start(out=tkt, in_=tick_d[:, :])
            wt = consts.tile([P, 4, D], f16)      # W16
            ct = consts.tile([P, 4, D], f16)      # C16
            idt = consts.tile([P, P], f16)        # identity for PE transpose
            nc.sync.dma_start(out=wt, in_=w_t[:, :, :])
            nc.sync.dma_start(out=ct, in_=c_t[:, :, :])
            nc.sync.dma_start(out=idt, in_=i_t[:, :])

            # vsh[p] = v[row 64p+64] ; wraps at p in {31,63,95,127} <- batch starts
            vsh = consts.tile([P, D], f16)
            vflat = v_d
            nc.gpsimd.dma_start(
                out=vsh[0:127], in_=vflat.rearrange("(a b) c -> a b c", b=NSUB)[1:128, 0]
            )
            nc.gpsimd.dma_start(
                out=vsh.rearrange("(w u) c -> w u c", u=32)[:, 31:32, :].rearrange("w u c -> (w u) c"),
                in_=vflat.rearrange("(b t) c -> b t c", t=L)[:, 0:1, :].rearrange("b t c -> (b t) c"),
            )

            def load_super(sbi):
                sl = slice(sbi * SB_GROUP, (sbi + 1) * SB_GROUP)
                q16 = io.tile([P, SB_GROUP, D], f16, tag="q16")
                k16 = io.tile([P, SB_GROUP, D], f16, tag="k16")
                v16 = io.tile([P, SB_GROUP, D], f16, tag="v16")
                nc.gpsimd.dma_start(out=q16, in_=qv[:, sl, :])
                nc.gpsimd.dma_start(out=k16, in_=kv[:, sl, :])
                nc.gpsimd.dma_start(out=v16, in_=vv[:, sl, :])
                return q16, k16, v16

            def compute_half(q16, k16, hh, w1sb):
                """hh in {0,1}: subblocks u0=4*hh .. u0+3 (512 rows).

                xpose chunks (t, u, jj) on PE -> ptx PSUM f16 -> qkT SBUF;
                GEMM-1 4-jj-accum chains N=512 -> psA/psB -> qkf SBUF f16;
                products on DVE -> pt; GEMM-2 4-ff chains N=512 per subblock;
                max8 + sigmoid -> w1sb[:, u, :].
                """
                u0 = 4 * hh
                # ---- transpose q,k chunks: ptx tile = (jj-half jh: 2 jj) x 4 u
                # q and k tiles are live together and their chunk-mms interleave
                # so consecutive PE writes land in different PSUM banks (drains
                # overlap: ~65-90ns/mm instead of ~169ns same-bank).
                qkT = work.tile([P, 2, 4, 4, P], f16, tag="qkT", bufs=3)  # [t, jj, u, 128]
                for jh in range(2):
                    ptxq = psxp.tile([P, 8, P], f16, tag="ptx")
                    ptxk = psxp.tile([P, 8, P], f16, tag="ptx")
                    for u in range(4):
                        for j2 in range(2):
                            jj = 2 * jh + j2
                            nc.tensor.transpose(
                                ptxq[:, 4 * j2 + u, :],
                                q16[:, u0 + u, jj * P:(jj + 1) * P], idt)
                            nc.tensor.transpose(
                                ptxk[:, 4 * j2 + u, :],
                                k16[:, u0 + u, jj * P:(jj + 1) * P], idt)
                    for t, ptx in ((0, ptxq), (1, ptxk)):
                        dst = qkT[:, t, 2 * jh:2 * jh + 2, :, :]
                        if (t + jh) % 2 == 0:
                            nc.scalar.copy(dst, ptx.rearrange("p (a b) c -> p a b c", a=2))
                        else:
                            nc.vector.tensor_copy(dst, ptx.rearrange("p (a b) c -> p a b c", a=2))

                # ---- GEMM-1: chain over jj accumulating, N=512, 1-bank tiles
                qkf = work.tile([P, 2, 2, 2, D], f16, tag="qkf", bufs=3)  # [t, ab, m2, 512]
                for t in range(2):
                    for ab in range(2):
                        for m2 in range(2):
                            mm = 2 * ab + m2
                            ps = psg1.tile([P, D], f32, tag="g1")
                            for jj in range(4):
                                nc.tensor.matmul(
                                    ps,
                                    wt[:, jj, mm * P:(mm + 1) * P],
                                    qkT[:, t, jj, :, :],
                                    start=(jj == 0), stop=(jj == 3))
                            dst = qkf[:, t, ab, m2, :]
                            if (t + ab + m2) % 2 == 0:
                                nc.scalar.copy(dst, ps)
                            else:
                                nc.vector.tensor_copy(dst, ps)

                # ---- products: Pa = QA.KA + QB.KB ; Pb = QB.KA - QA.KB
                pt = work.tile([P, 4, D], f16, tag="pt", bufs=3)       # [mm, 512 rows]
                t1 = work.tile([P, 2, D], f16, tag="t1")
                t2 = work.tile([P, 2, D], f16, tag="t2")
                QA, QB = qkf[:, 0, 0, :, :], qkf[:, 0, 1, :, :]
                KA, KB = qkf[:, 1, 0, :, :], qkf[:, 1, 1, :, :]
                nc.vector.tensor_mul(t1, QA, KA)
                nc.vector.tensor_mul(t2, QB, KB)
                nc.vector.tensor_add(pt[:, 0:2, :], t1, t2)
                nc.vector.tensor_mul(t1, QB, KA)
                nc.vector.tensor_mul(t2, QA, KB)
                nc.vector.tensor_sub(pt[:, 2:4, :], t1, t2)
                # f=0 fixup: partition 0 of mm0 (Re0) and mm2 (Re256)
                nc.vector.tensor_mul(
                    pt[0:1, 0:4:2, :], qkf[0:1, 0, :, 0, :], qkf[0:1, 1, :, 0, :])

                # ---- GEMM-2 + top7-mean + sigmoid per subblock (128 rows)
                for u in range(4):
                    cps = psg2.tile([P, D], f32, tag="g2")
                    for ff in range(4):
                        nc.tensor.matmul(cps, pt[:, ff, u * P:(u + 1) * P],
                                         ct[:, ff, :], start=(ff == 0), stop=(ff == 3))
                    mx = small.tile([P, 8], f32, tag="mx")
                    nc.vector.max(out=mx, in_=cps)
                    sm = small.tile([P, 1], f32, tag="sm")
                    nc.vector.reduce_sum(sm, mx[:, 0:TOPK], axis=mybir.AxisListType.X)
                    pm = small.tile([P, 1], f32, tag="pm")
                    nc.vector.tensor_scalar_mul(pm, sm, 1.0 / TOPK)
                    nc.scalar.activation(w1sb[:, u0 + u, :], cps,
                                         mybir.ActivationFunctionType.Sigmoid,
                                         bias=pm, scale=-1.0)

            def combine_super(v16, w1sb, vnext0, o16):
                """o16 = v16 + w1sb*(roll(v16) - v16), wide ops."""
                dtw = work.tile([P, SB_GROUP, D], f16, tag="dtw")
                ztw = work.tile([P, SB_GROUP, D], f16, tag="ztw")
                nc.vector.tensor_sub(dtw[:, 0:7, :], v16[:, 1:8, :], v16[:, 0:7, :])
                nc.vector.tensor_sub(dtw[:, 7, :], vnext0, v16[:, 7, :])
                # keep Pool free: it must pump the casting load DMAs
                nc.vector.tensor_mul(ztw, w1sb, dtw)
                nc.vector.tensor_add(o16, v16, ztw)

            def pipeline():
                prev = None  # (v16, o16, w1sb, sbi)
                for sbi in range(NSUPER):
                    q16, k16, v16 = load_super(sbi)
                    o16 = io.tile([P, SB_GROUP, D], f16, tag="o16")
                    w1sb = work.tile([P, SB_GROUP, D], f16, tag="w1sb")
                    for hh in range(2):
                        compute_half(q16, k16, hh, w1sb)
                    if prev is not None:
                        pv, po, pw, psbi = prev
                        combine_super(pv, pw, v16[:, 0, :], po)
                        nc.sync.dma_start(
                            out=ov[:, psbi * SB_GROUP:(psbi + 1) * SB_GROUP, :], in_=po)
                    prev = (v16, o16, w1sb, sbi)

                pv, po, pw, psbi = prev
                combine_super(pv, pw, vsh, po)
                nc.sync.dma_start(
                    out=ov[:, psbi * SB_GROUP:(psbi + 1) * SB_GROUP, :], in_=po)

            if n_iter == 1:
                pipeline()
            else:
                with tc.For_i(0, n_iter, 1):
                    pipeline()

            if internal_io:
                nc.sync.dma_start(out=tock_d[:, :], in_=tkt)

    nc.finalize()
    return nc


def kernel(query, key, value):
    import sys
    if "/opt/trn_rl_repo" not in sys.path:
        sys.path.insert(0, "/opt/trn_rl_repo")
    from concourse.bass_utils import run_bass_kernel_spmd

    if "nc" not in _CACHE:
        _CACHE["nc"] = _build_nc()
    nc = _CACHE["nc"]

    q = np.ascontiguousarray(np.asarray(query, dtype=np.float32).reshape(B, L, D))
    k = np.ascontiguousarray(np.asarray(key, dtype=np.float32).reshape(B, L, D))
    v = np.ascontiguousarray(np.asarray(value, dtype=np.float32).reshape(B, L, D))

    in_maps = []
    for c in range(N_CORES):
        sl = slice(c * BPC, (c + 1) * BPC)
        in_maps.append({
            "query": q[sl].reshape(ROWS, D),
            "key": k[sl].reshape(ROWS, D),
            "value": v[sl].reshape(ROWS, D),
        })
    res = run_bass_kernel_spmd(nc, in_maps, core_ids=list(range(N_CORES)))
    _CACHE["last_result"] = res
    out = np.empty((B, L, D), dtype=np.float32)
    for c in range(N_CORES):
        out[c * BPC:(c + 1) * BPC] = res.results[c]["out"].astype(np.float32).reshape(BPC, L, D)
    return out
